# revision 1
# baseline (speedup 1.0000x reference)
"""Trainium2 Bass kernel for nn_CAMLoss.

Data-parallel over batch across 8 NeuronCores (8 samples/core); the final
scalar mean is combined with an on-device AllReduce.

Math refactoring (validated to ~3e-7 rel err vs the JAX reference on CPU):
for each sample with features f[c,a,b] (c=2048 channels, a,b in 14x14):
  - cam_t[i,j] = sum_c w3[t,c] f[c,i,j]; normalized to [0,255]
  - fea0-feat = D_t @ f_c with D_t = cam0n - camtn  (per channel c)
  - ||fea0-feat||^2 = sum_{a,a'} C_t[a,a'] G[a,a']  where C_t = D_t^T D_t and
    G[a,a'] = sum_{c,b} f[c,a,b] f[c,a',b]
G is recovered from the channel Gram matrix M = F^T F (PE-friendly: contraction
over c in 128-chunks) by summing its b-diagonal blocks.  The +eps inside the
big pairwise distance shifts sumsq by ~1e-11 relative and is dropped; the eps
in the seg-distance is kept exactly.

One PE pass per sample computes both M (rows ordered (b,a) so the diagonal
blocks are partition-contiguous) and the three CAM rows (w3^T fused as extra
lhsT columns).  Features are read from HBM exactly once -> memory roofline.
"""

import numpy as np
from contextlib import ExitStack

BZ, NCH, H, W_SP, NCLS = 64, 2048, 14, 14, 1000
NCORES = 8
SH = BZ // NCORES            # samples per core
HW = H * W_SP                # 196
P = 128
NCHUNK = NCH // P            # 16
MARGIN, THR, PD_EPS = 70.0, 125.0, 1e-6

_CACHE: dict = {}


def _build(collective=True, stage=5):
    import concourse.bass as bass
    import concourse.tile as tile
    from concourse import bacc, mybir
    from concourse.masks import make_identity

    f32 = mybir.dt.float32
    bf16 = mybir.dt.bfloat16
    i32 = mybir.dt.int32
    Alu = mybir.AluOpType
    Act = mybir.ActivationFunctionType
    Ax = mybir.AxisListType

    nc = bacc.Bacc(None, target_bir_lowering=False)
    feats = nc.declare_dram_parameter("feats", [SH, NCH, H, W_SP], f32, isOutput=False)
    pred = nc.declare_dram_parameter("pred", [SH, NCLS], f32, isOutput=False)
    seg = nc.declare_dram_parameter("seg", [SH, HW], f32, isOutput=False)
    cla = nc.declare_dram_parameter("cla", [SH, 1], i32, isOutput=False)
    idx = nc.declare_dram_parameter("idx", [3 * SH, 1], i32, isOutput=False)
    wsm = nc.declare_dram_parameter("wsm", [NCLS, NCH], f32, isOutput=False)
    out_ext = nc.declare_dram_parameter("out", [1, 1], f32, isOutput=True)

    cc_in = nc.dram_tensor("cc_in", [1, 1], f32)
    cc_out = nc.dram_tensor("cc_out", [1, 1], f32, addr_space="Shared")

    NW = 3 * SH  # gathered weight rows (24)

    with ExitStack() as ctx:
        tc = ctx.enter_context(tile.TileContext(nc))
        singles = ctx.enter_context(tc.tile_pool(name="singles", bufs=1))
        fpool = ctx.enter_context(tc.tile_pool(name="fpool", bufs=2))
        l2pool = ctx.enter_context(tc.tile_pool(name="l2pool", bufs=2))
        gpool = ctx.enter_context(tc.tile_pool(name="gpool", bufs=2))
        ma_pool = ctx.enter_context(tc.tile_pool(name="ma", bufs=2, space="PSUM"))
        mb_pool = ctx.enter_context(tc.tile_pool(name="mb", bufs=2, space="PSUM"))
        tp_pool = ctx.enter_context(tc.tile_pool(name="tp", bufs=1, space="PSUM"))
        c_pool = ctx.enter_context(tc.tile_pool(name="cp", bufs=2, space="PSUM"))
        fs_pool = ctx.enter_context(tc.tile_pool(name="fs", bufs=1, space="PSUM"))

        # ---- gather the 24 needed weight_softmax rows, build w3T [128,16,24]
        idx_sb = singles.tile([NW, 1], i32)
        nc.sync.dma_start(out=idx_sb[:], in_=idx[:])
        w_sel = singles.tile([NW, NCH], f32)
        nc.gpsimd.indirect_dma_start(
            out=w_sel[:],
            out_offset=None,
            in_=wsm[:],
            in_offset=bass.IndirectOffsetOnAxis(ap=idx_sb[:, :1], axis=0),
        )
        ident = singles.tile([P, P], f32)
        make_identity(nc, ident[:])
        # dummy PE op reading only ident: absorbs the gpsimd semaphore so the
        # real transposes carry a single wait (LDWEIGHTS wait-slot limit)
        tpd = tp_pool.tile([NW, NW], f32, tag="tp")
        nc.tensor.transpose(
            out=tpd[:], in_=ident[:NW, :NW], identity=ident[:NW, :NW]
        )
        # w3t[p, ci, t] = w3[t, c] with c = p*16 + ci (same mapping as f_sb)
        w3t = singles.tile([P, NCHUNK, NW], bf16)
        w_sel_v = w_sel[:].rearrange("w (x ci) -> w x ci", ci=NCHUNK)
        for ci in range(NCHUNK):
            tp = tp_pool.tile([P, NW], f32, tag="tp")
            nc.tensor.transpose(
                out=tp[:], in_=w_sel_v[:, :, ci], identity=ident[:NW, :NW]
            )
            nc.scalar.copy(out=w3t[:, ci, :], in_=tp[:])

        # ---- per-sample PE pass: M = F^T F (rows (b,a)-ordered) + cam rows
        # cam rows live quadrant-aligned: cam_t for sample s at partition 32t+s
        cams = singles.tile([96, HW], f32)
        nc.gpsimd.memset(cams[:], 0.0)
        gall = singles.tile([14, SH * 14], f32)  # per-sample G side by side
        evac_pool = ctx.enter_context(tc.tile_pool(name="evac", bufs=1))
        gd_pool = ctx.enter_context(tc.tile_pool(name="gd", bufs=1))
        ma_all = evac_pool.tile([126, SH, HW], f32, tag="ma_all")
        mb_all = evac_pool.tile([73, SH, HW], f32, tag="mb_all")
        for s in range(SH):
            # channel mapping c = p*16 + ch keeps the HBM read fully sequential
            f_sb = fpool.tile([P, NCHUNK, HW], f32)
            nc.sync.dma_start(
                out=f_sb[:],
                in_=feats[s].rearrange("(p ch) h w -> p ch (h w)", ch=NCHUNK),
            )
            # lhsT assembled in (b,a) column order so M rows come out
            # (b,a)-ordered: then each b-diagonal block is a contiguous
            # 14-partition range (matmul weight APs must be 2D, and DMA
            # cannot do partition-strided SBUF reads).
            lall = l2pool.tile([P, NCHUNK, 199], bf16)
            f_ba = f_sb[:].rearrange("p ch (a b) -> p ch b a", b=14)
            lhw = lall[:, :, 0:HW].rearrange("p ch (b a) -> p ch b a", a=14)
            nc.vector.tensor_copy(out=lhw[:, 0:9], in_=f_ba[:, 0:9])
            nc.scalar.copy(out=lhw[:, 9:], in_=f_ba[:, 9:])
            nc.vector.tensor_copy(
                out=lall[:, :, HW:HW + 3], in_=w3t[:, :, 3 * s:3 * s + 3]
            )

            ma = ma_pool.tile([126, HW], f32)    # M rows (b,a), b=0..8
            mb = mb_pool.tile([73, HW], f32)     # M rows b=9..13 + 3 cam rows
            for ci in range(NCHUNK):
                st, sp = ci == 0, ci == NCHUNK - 1
                nc.tensor.matmul(
                    ma[:], lall[:, ci, 0:126], lall[:, ci, 0:HW], start=st, stop=sp
                )
                nc.tensor.matmul(
                    mb[:], lall[:, ci, 126:199], lall[:, ci, 0:HW], start=st, stop=sp
                )
            # evacuate M to SBUF, batched across samples (engines need
            # quadrant-aligned partition starts; DMA gathers below don't)
            nc.scalar.copy(out=ma_all[:, s, :], in_=ma[:])
            nc.vector.tensor_copy(out=mb_all[:, s, :], in_=mb[:])
            if stage <= 1:
                nc.sync.dma_start(out=out_ext[:], in_=mb_all[0:1, 0:1, 0:1])
                return nc

        # cam rows out to the quadrant-aligned cam tile; gathers are split
        # into sample-halves so the first half overlaps the main loop
        HH = SH // 2
        for t in range(3):
            nc.gpsimd.dma_start(
                out=cams[32 * t:32 * t + HH, :], in_=mb_all[70 + t:71 + t, 0:HH, :]
            )
            nc.gpsimd.dma_start(
                out=cams[32 * t + HH:32 * t + SH, :],
                in_=mb_all[70 + t:71 + t, HH:SH, :],
            )
        # G[a,a'] = sum_b M[(b,a), (a',b)]: one DMA per (b, sample-half)
        # gathers that diagonal block into gdiag[a, (s, b, x)], then a single
        # reduce over b produces every per-sample G at once.  (DMA APs: max 3
        # dims, contiguous innermost run; the reduce reads a strided view.)
        gdiag = gd_pool.tile([14, SH, 196], f32)
        for b in range(14):
            srct = ma_all if b < 9 else mb_all
            r0 = b * 14 if b < 9 else (b - 9) * 14
            eng = (nc.sync, nc.gpsimd, nc.scalar)[b % 3]
            for s0, s1 in ((0, HH), (HH, SH)):
                eng.dma_start(
                    out=gdiag[:, s0:s1, b * 14:(b + 1) * 14],
                    in_=srct[r0:r0 + 14, s0:s1, b * 14:(b + 1) * 14],
                )
        nc.vector.tensor_reduce(
            out=gall[:],
            in_=gdiag[:].rearrange("p s (b x) -> p s x b", x=14),
            axis=Ax.X, op=Alu.add,
        )
        if stage <= 2:
            nc.sync.dma_start(out=out_ext[:], in_=gall[0:1, 0:1])
            return nc

        # ---- batched CAM normalization: camn = (cam - min) / max(cam - min) * 255
        # rows 8..31 / 40..63 are zero padding; per-partition ops keep them inert
        mn = singles.tile([96, 1], f32)
        nc.vector.tensor_reduce(out=mn[:], in_=cams[:], axis=Ax.X, op=Alu.min)
        camsub = singles.tile([96, HW], f32)
        nc.vector.tensor_scalar(
            out=camsub[:], in0=cams[:], scalar1=mn[:], scalar2=None, op0=Alu.subtract
        )
        mx = singles.tile([96, 1], f32)
        nc.vector.tensor_reduce(out=mx[:], in_=camsub[:], axis=Ax.X, op=Alu.max)
        # keep the zero padding rows finite through the reciprocal
        nc.vector.tensor_scalar_max(out=mx[:], in0=mx[:], scalar1=1e-30)
        rmx = singles.tile([96, 1], f32)
        nc.vector.reciprocal(out=rmx[:], in_=mx[:])
        camn_wh = singles.tile([96, HW], f32)
        nc.vector.tensor_scalar(
            out=camn_wh[:], in0=camsub[:], scalar1=rmx[:], scalar2=255.0,
            op0=Alu.mult, op1=Alu.mult,
        )
        # cam rows came out (w,h)-ordered (matmul cols are lall-ordered);
        # one strided copy puts them in natural (h,w) order for everything
        # downstream (seg compare, row reduce, D reshape DMAs)
        camn = singles.tile([96, HW], f32)
        nc.vector.tensor_copy(
            out=camn[:].rearrange("p (h w) -> p h w", w=14),
            in_=camn_wh[:].rearrange("p (w h) -> p h w", h=14),
        )

        # ---- D_t = cam0n - camtn, reshaped to [14,14] per sample via tiny DMAs
        # (engine operands must share a partition range -> DMA-bounce the
        # cam1/cam2 quadrant blocks down to partitions 0..7 first)
        c1loc = singles.tile([SH, HW], f32)
        c2loc = singles.tile([SH, HW], f32)
        nc.sync.dma_start(out=c1loc[:], in_=camn[32:32 + SH, :])
        nc.sync.dma_start(out=c2loc[:], in_=camn[64:64 + SH, :])
        d1 = singles.tile([SH, HW], f32)
        d2 = singles.tile([SH, HW], f32)
        nc.vector.tensor_tensor(
            out=d1[:], in0=camn[0:SH, :], in1=c1loc[:], op=Alu.subtract
        )
        nc.vector.tensor_tensor(
            out=d2[:], in0=camn[0:SH, :], in1=c2loc[:], op=Alu.subtract
        )
        dmats = singles.tile([14, 2 * SH * 14], f32)
        dma_engs = (nc.sync, nc.gpsimd, nc.scalar)
        for t, dt_tile in enumerate((d1, d2)):
            for s in range(SH):
                dma_engs[(t * SH + s) % 3].dma_start(
                    out=dmats[:, (t * SH + s) * 14:(t * SH + s + 1) * 14],
                    in_=dt_tile[s:s + 1, :].rearrange("p (i a) -> p i a", a=14),
                )

        if stage <= 3:
            nc.sync.dma_start(out=out_ext[:], in_=dmats[0:1, 0:1])
            return nc

        # ---- ed1 (row-wise distance of binarized cam0 to seg truth)
        seg_sb = singles.tile([SH, HW], f32)
        nc.gpsimd.dma_start(out=seg_sb[:], in_=seg[:])
        x = singles.tile([SH, HW], f32)
        nc.vector.scalar_tensor_tensor(
            out=x[:], in0=camn[0:SH, :], scalar=THR, in1=seg_sb[:],
            op0=Alu.is_gt, op1=Alu.subtract,
        )  # x = (cam0n > THR) - seg
        eps_c = singles.tile([SH, 1], f32)
        nc.gpsimd.memset(eps_c[:], PD_EPS)
        xx = singles.tile([SH, HW], f32)
        nc.scalar.activation(out=xx[:], in_=x[:], func=Act.Square, bias=eps_c[:])
        r2 = singles.tile([SH, 14], f32)
        nc.vector.tensor_reduce(
            out=r2[:], in_=xx[:].rearrange("p (i a) -> p i a", a=14),
            axis=Ax.X, op=Alu.add,
        )
        rr = singles.tile([SH, 14], f32)
        nc.scalar.sqrt(rr[:], r2[:])
        ed1s = singles.tile([SH, 1], f32)
        nc.vector.tensor_reduce(out=ed1s[:], in_=rr[:], axis=Ax.X, op=Alu.add)

        # ---- cross entropy: lse(pred) - pred[cla]
        pred_sb = singles.tile([SH, NCLS], f32)
        nc.gpsimd.dma_start(out=pred_sb[:], in_=pred[:])
        cla_sb = singles.tile([SH, 1], i32)
        nc.gpsimd.dma_start(out=cla_sb[:], in_=cla[:])
        iot = singles.tile([SH, NCLS], f32)
        nc.gpsimd.iota(
            out=iot[:], pattern=[[1, NCLS]], base=0, channel_multiplier=0,
            allow_small_or_imprecise_dtypes=True,
        )
        cla_f = singles.tile([SH, 1], f32)
        nc.vector.tensor_copy(out=cla_f[:], in_=cla_sb[:])
        onehot = singles.tile([SH, NCLS], f32)
        nc.vector.tensor_scalar(
            out=onehot[:], in0=iot[:], scalar1=cla_f[:], scalar2=None,
            op0=Alu.is_equal,
        )
        scr1k = singles.tile([SH, NCLS], f32)
        nc.vector.tensor_mul(out=scr1k[:], in0=onehot[:], in1=pred_sb[:])
        tgt = singles.tile([SH, 1], f32)
        nc.vector.tensor_reduce(out=tgt[:], in_=scr1k[:], axis=Ax.X, op=Alu.add)
        pmax = singles.tile([SH, 1], f32)
        nc.vector.tensor_reduce(out=pmax[:], in_=pred_sb[:], axis=Ax.X, op=Alu.max)
        negm = singles.tile([SH, 1], f32)
        nc.vector.tensor_scalar(
            out=negm[:], in0=pmax[:], scalar1=-1.0, scalar2=None, op0=Alu.mult
        )
        esc = singles.tile([SH, NCLS], f32)
        sume = singles.tile([SH, 1], f32)
        nc.scalar.activation(
            out=esc[:], in_=pred_sb[:], func=Act.Exp, bias=negm[:], scale=1.0,
            accum_out=sume[:],
        )
        lns = singles.tile([SH, 1], f32)
        nc.scalar.activation(out=lns[:], in_=sume[:], func=Act.Ln)
        ce = singles.tile([SH, 1], f32)
        nc.vector.tensor_add(out=ce[:], in0=pmax[:], in1=lns[:])
        nc.vector.tensor_sub(out=ce[:], in0=ce[:], in1=tgt[:])

        # v = ed1s/14 + ce   (per-sample CE + seg-distance contribution)
        v = singles.tile([SH, 1], f32)
        nc.vector.scalar_tensor_tensor(
            out=v[:], in0=ed1s[:], scalar=1.0 / 14.0, in1=ce[:],
            op0=Alu.mult, op1=Alu.add,
        )

        if stage == 35:
            nc.sync.dma_start(out=out_ext[:], in_=v[0:1, 0:1])
            return nc

        # ---- acc columns: [2s]=sumsq1, [2s+1]=sumsq2, [16]=v (padded)
        acc = singles.tile([14, 2 * SH + 1], f32)
        nc.gpsimd.memset(acc[0:14, 2 * SH:2 * SH + 1], 0.0)
        nc.scalar.copy(out=acc[0:SH, 2 * SH:2 * SH + 1], in_=v[:])
        scr14 = singles.tile([14, 2 * SH * 14], f32)
        for s in range(SH):
            for t in range(2):
                k = 2 * s + t
                cps = c_pool.tile([14, 14], f32)
                dsl = dmats[:, (t * SH + s) * 14:(t * SH + s + 1) * 14]
                nc.tensor.matmul(cps[:], dsl, dsl, start=True, stop=True)
                # acc[:, k] = sum_x C[:, x] * G[:, x]  (fused mul+row-sum)
                nc.vector.scalar_tensor_tensor(
                    out=scr14[:, k * 14:(k + 1) * 14], in0=cps[:], scalar=0.0,
                    in1=gall[:, s * 14:(s + 1) * 14], op0=Alu.add, op1=Alu.mult,
                    accum_out=acc[:, k:k + 1],
                )

        if stage <= 4:
            nc.sync.dma_start(out=out_ext[:], in_=acc[0:1, 0:1])
            return nc

        # ---- partition-reduce acc via ones-matmul, then the scalar tail
        ones = singles.tile([14, 1], f32)
        nc.gpsimd.memset(ones[:], 1.0)
        fs = fs_pool.tile([1, 2 * SH + 1], f32)
        nc.tensor.matmul(fs[:], ones[:], acc[:], start=True, stop=True)
        dvals = singles.tile([1, 2 * SH], f32)
        nc.scalar.activation(
            out=dvals[:], in_=fs[0:1, 0:2 * SH], func=Act.Sqrt,
            scale=1.0 / float(NCH) ** 2,
        )
        dv = dvals[:].rearrange("p (s t) -> p s t", t=2)
        dsum = singles.tile([1, SH], f32)
        nc.vector.tensor_tensor(out=dsum[:], in0=dv[:, :, 0], in1=dv[:, :, 1], op=Alu.add)
        marg_c = singles.tile([1, 1], f32)
        nc.gpsimd.memset(marg_c[:], MARGIN)
        relu_z = singles.tile([1, SH], f32)
        nc.scalar.activation(
            out=relu_z[:], in_=dsum[:], func=Act.Relu, bias=marg_c[:], scale=-1.0
        )
        rz = singles.tile([1, 1], f32)
        nc.vector.tensor_reduce(out=rz[:], in_=relu_z[:], axis=Ax.X, op=Alu.add)
        tot = singles.tile([1, 1], f32)
        nc.vector.tensor_add(out=tot[:], in0=rz[:], in1=fs[0:1, 2 * SH:2 * SH + 1])
        partial = singles.tile([1, 1], f32)
        nc.vector.tensor_scalar(
            out=partial[:], in0=tot[:], scalar1=1.0 / float(BZ), scalar2=None,
            op0=Alu.mult,
        )

        # ---- AllReduce the partial means, write the final scalar
        if collective:
            nc.sync.dma_start(out=cc_in[:], in_=partial[:])
            nc.gpsimd.collective_compute(
                "AllReduce",
                mybir.AluOpType.add,
                replica_groups=[list(range(NCORES))],
                ins=[cc_in[:]],
                outs=[cc_out[:]],
            )
            final_sb = singles.tile([1, 1], f32)
            nc.sync.dma_start(out=final_sb[:], in_=cc_out[:])
            nc.sync.dma_start(out=out_ext[:], in_=final_sb[:])
        else:
            nc.sync.dma_start(out=out_ext[:], in_=partial[:])

    return nc


USE_COLLECTIVE = True


def kernel(pred, cla_truth, seg_truth, features_blobs, weight_softmax, idx,
           _trace=False, _tmpdir=None):
    from concourse.bass_utils import run_bass_kernel_spmd

    if "nc" not in _CACHE:
        nc = _build(collective=USE_COLLECTIVE)
        if not nc.is_finalized():
            nc.finalize()
        _CACHE["nc"] = nc
    nc = _CACHE["nc"]

    pred = np.ascontiguousarray(np.asarray(pred, dtype=np.float32))
    cla = np.ascontiguousarray(np.asarray(cla_truth, dtype=np.int32))
    seg = np.ascontiguousarray(np.asarray(seg_truth, dtype=np.float32))
    feats = np.ascontiguousarray(np.asarray(features_blobs, dtype=np.float32))
    wsm = np.ascontiguousarray(np.asarray(weight_softmax, dtype=np.float32))
    idx = np.ascontiguousarray(np.asarray(idx, dtype=np.int32))

    in_maps = []
    for r in range(NCORES):
        sl = slice(r * SH, (r + 1) * SH)
        in_maps.append({
            "feats": np.ascontiguousarray(feats[sl]),
            "pred": np.ascontiguousarray(pred[sl]),
            "seg": np.ascontiguousarray(seg[sl].reshape(SH, HW)),
            "cla": np.ascontiguousarray(cla[sl].reshape(SH, 1)),
            "idx": np.ascontiguousarray(idx[sl].reshape(3 * SH, 1)),
            "wsm": wsm,
        })

    res = run_bass_kernel_spmd(
        nc, in_maps, list(range(NCORES)), trace=_trace, tmpdir=_tmpdir
    )
    if _trace:
        _CACHE["last_results"] = res
    if USE_COLLECTIVE:
        val = np.asarray(res.results[0]["out"]).reshape(())
    else:
        val = np.sum([np.asarray(r["out"]).reshape(()) for r in res.results],
                     dtype=np.float32)
    return np.float32(val)



# revision 5
# speedup vs baseline: 1.6102x; 1.6102x over previous
"""Trainium2 Bass kernel for nn_CAMLoss.

Data-parallel over batch across 8 NeuronCores (8 samples/core); the per-core
partial means are summed on the host at gather time (an on-device AllReduce
of the final scalar costs ~25us of pure latency).

Math refactoring (validated to ~3e-7 rel err vs the JAX reference on CPU):
for each sample with features f[c,h,w] (c=2048 channels, h,w in 14x14):
  - cam_t[i,j] = sum_c w3[t,c] f[c,i,j]; normalized to [0,255]
  - fea0-feat = D_t @ f_c with D_t = cam0n - camtn  (per channel c)
  - ||fea0-feat||^2 = sum_{k,k'} C_t[k,k'] G[k,k']  where C_t = D_t^T D_t and
    G[k,k'] = sum_{c,w} f[c,k,w] f[c,k',w]
G is recovered from the channel Gram matrix M = F^T F (PE-friendly: contraction
over c in 128-chunks) by summing its w-diagonal blocks.  The +eps inside the
big pairwise distance shifts sumsq by ~1e-11 relative and is dropped; the eps
in the seg-distance is kept exactly.

One PE pass per sample computes both M (rows ordered (w,h) so the diagonal
blocks are partition-contiguous) and the three CAM rows (w3^T fused as extra
lhsT columns).  The pass is split 112/87 rows so the first matmul only
streams the 112 rhs columns its diagonal blocks need (308 moving cols/chunk
instead of 392).  Features are read from HBM exactly once -> memory roofline.
"""

import numpy as np
from contextlib import ExitStack

BZ, NCH, H, W_SP, NCLS = 64, 2048, 14, 14, 1000
NCORES = 8
SH = BZ // NCORES            # samples per core
HW = H * W_SP                # 196
P = 128
NCHUNK = NCH // P            # 16
MARGIN, THR, PD_EPS = 70.0, 125.0, 1e-6

_CACHE: dict = {}


def _build(collective=False):
    import concourse.bass as bass
    import concourse.tile as tile
    from concourse import bacc, mybir
    from concourse.masks import make_identity

    f32 = mybir.dt.float32
    bf16 = mybir.dt.bfloat16
    i32 = mybir.dt.int32
    Alu = mybir.AluOpType
    Act = mybir.ActivationFunctionType
    Ax = mybir.AxisListType

    nc = bacc.Bacc(None, target_bir_lowering=False)
    feats = nc.declare_dram_parameter("feats", [SH, NCH, H, W_SP], f32, isOutput=False)
    pred = nc.declare_dram_parameter("pred", [SH, NCLS], f32, isOutput=False)
    seg = nc.declare_dram_parameter("seg", [SH, HW], f32, isOutput=False)
    cla = nc.declare_dram_parameter("cla", [SH, 1], i32, isOutput=False)
    idx = nc.declare_dram_parameter("idx", [3 * SH, 1], i32, isOutput=False)
    wsm = nc.declare_dram_parameter("wsm", [NCLS, NCH], f32, isOutput=False)
    out_ext = nc.declare_dram_parameter("out", [1, 1], f32, isOutput=True)

    if collective:
        cc_in = nc.dram_tensor("cc_in", [1, 1], f32)
        cc_out = nc.dram_tensor("cc_out", [1, 1], f32, addr_space="Shared")

    NW = 3 * SH  # gathered weight rows (24)
    RA = 112     # ma rows/cols: diagonal blocks b=0..7
    RB = 87      # mb rows: blocks 8..13 (84) + 3 cam rows

    with ExitStack() as ctx:
        tc = ctx.enter_context(tile.TileContext(nc))
        singles = ctx.enter_context(tc.tile_pool(name="singles", bufs=1))
        fpool = ctx.enter_context(tc.tile_pool(name="fpool", bufs=3))
        l2pool = ctx.enter_context(tc.tile_pool(name="l2pool", bufs=2))
        ma_pool = ctx.enter_context(tc.tile_pool(name="ma", bufs=2, space="PSUM"))
        mb_pool = ctx.enter_context(tc.tile_pool(name="mb", bufs=2, space="PSUM"))
        tp_pool = ctx.enter_context(tc.tile_pool(name="tp", bufs=1, space="PSUM"))
        c_pool = ctx.enter_context(tc.tile_pool(name="cp", bufs=1, space="PSUM"))
        fs_pool = ctx.enter_context(tc.tile_pool(name="fs", bufs=1, space="PSUM"))
        evac_pool = ctx.enter_context(tc.tile_pool(name="evac", bufs=1))
        gd_pool = ctx.enter_context(tc.tile_pool(name="gd", bufs=1))

        # ---- feats DMA for sample 0 is the first instruction on the sync
        # queue: everything else (gathers, CE loads) rides the gpsimd queue
        # so the HBM stream starts immediately.
        f_tiles = []
        f0 = fpool.tile([P, NCHUNK, HW], f32)
        fv0 = feats[0].rearrange("(p ch) h w -> p ch (h w)", ch=NCHUNK)
        nc.sync.dma_start(out=f0[:, 0:NCHUNK // 2], in_=fv0[:, 0:NCHUNK // 2])
        nc.sync.dma_start(out=f0[:, NCHUNK // 2:], in_=fv0[:, NCHUNK // 2:])
        f_tiles.append(f0)

        # ---- gather the 24 needed weight_softmax rows, build w3T [128,16,24]
        idx_sb = singles.tile([NW, 1], i32)
        nc.gpsimd.dma_start(out=idx_sb[:], in_=idx[:])
        w_sel = singles.tile([NW, NCH], f32)
        nc.gpsimd.indirect_dma_start(
            out=w_sel[:],
            out_offset=None,
            in_=wsm[:],
            in_offset=bass.IndirectOffsetOnAxis(ap=idx_sb[:, :1], axis=0),
        )
        ident = singles.tile([P, P], f32)
        make_identity(nc, ident[:])
        # dummy PE op reading only ident: absorbs the gpsimd semaphore so the
        # real transposes carry a single wait (LDWEIGHTS wait-slot limit)
        tpd = tp_pool.tile([NW, NW], f32, tag="tp")
        nc.tensor.transpose(
            out=tpd[:], in_=ident[:NW, :NW], identity=ident[:NW, :NW]
        )
        # w3t[p, ci, t] = w3[t, c] with c = p*16 + ci (same mapping as f_sb)
        w3t = singles.tile([P, NCHUNK, NW], bf16)
        w_sel_v = w_sel[:].rearrange("w (x ci) -> w x ci", ci=NCHUNK)
        for ci in range(NCHUNK):
            tp = tp_pool.tile([P, NW], f32, tag="tp")
            nc.tensor.transpose(
                out=tp[:], in_=w_sel_v[:, :, ci], identity=ident[:NW, :NW]
            )
            nc.scalar.copy(out=w3t[:, ci, :], in_=tp[:])

        # cam rows live quadrant-aligned: cam_t for sample s at partition 32t+s
        cams = singles.tile([96, HW], f32)
        nc.gpsimd.memset(cams[:], 0.0)

        # ---- per-sample PE pass: M = F^T F (rows (w,h)-ordered) + cam rows
        ma_all = evac_pool.tile([RA, SH, RA], f32, tag="ma_all")
        mb_all = evac_pool.tile([RB, SH, HW], f32, tag="mb_all")
        HH = SH // 2
        gdiag = gd_pool.tile([14, SH, 196], f32)
        gather_engs = (nc.sync, nc.gpsimd)

        def emit_gdiag(s0, s1, qi0):
            # G[k,k'] = sum_b M[(b,k), (k',b)]: one DMA per (b, sample-range)
            # gathers that diagonal block into gdiag[k, (s, b, x)]; a strided
            # reduce over b later produces every per-sample G at once.
            for b in range(14):
                srct = ma_all if b < 8 else mb_all
                r0 = b * 14 if b < 8 else (b - 8) * 14
                eng = gather_engs[(qi0 + b) % len(gather_engs)]
                eng.dma_start(
                    out=gdiag[:, s0:s1, b * 14:(b + 1) * 14],
                    in_=srct[r0:r0 + 14, s0:s1, b * 14:(b + 1) * 14],
                )

        def emit_cams(s0, s1):
            # cam rows t=0,1,2 out to partitions 32t+s in one DMA
            nc.gpsimd.dma_start(
                out=cams[:].rearrange("(t u) x -> t u x", u=32)[:, s0:s1, :],
                in_=mb_all[84:87, s0:s1, :],
            )

        for s in range(SH):
            # channel mapping c = p*16 + ch keeps the HBM read fully sequential
            if s == 0:
                f_sb = f_tiles[0]
            else:
                f_sb = fpool.tile([P, NCHUNK, HW], f32)
                fv = feats[s].rearrange("(p ch) h w -> p ch (h w)", ch=NCHUNK)
                nc.sync.dma_start(out=f_sb[:, 0:NCHUNK // 2], in_=fv[:, 0:NCHUNK // 2])
                nc.sync.dma_start(out=f_sb[:, NCHUNK // 2:], in_=fv[:, NCHUNK // 2:])
            # lhsT assembled in (w,h) column order so M rows come out
            # (w,h)-ordered: then each w-diagonal block is a contiguous
            # 14-partition range (matmul weight APs must be 2D, and DMA
            # cannot do partition-strided SBUF reads).  bf16 cast is fused
            # into the copy; per-DMA-half so the copy starts early; 10/16
            # chunks on vector, 6/16 on scalar (vector is ~2x faster).
            lall = l2pool.tile([P, NCHUNK, 199], bf16)
            f_ba = f_sb[:].rearrange("p ch (a b) -> p ch b a", b=14)
            lhw = lall[:, :, 0:HW].rearrange("p ch (b a) -> p ch b a", a=14)
            nc.vector.tensor_copy(out=lhw[:, 0:5], in_=f_ba[:, 0:5])
            nc.scalar.copy(out=lhw[:, 5:8], in_=f_ba[:, 5:8])
            nc.vector.tensor_copy(out=lhw[:, 8:13], in_=f_ba[:, 8:13])
            nc.scalar.copy(out=lhw[:, 13:16], in_=f_ba[:, 13:16])
            nc.vector.tensor_copy(
                out=lall[:, :, HW:HW + 3], in_=w3t[:, :, 3 * s:3 * s + 3]
            )

            ma = ma_pool.tile([RA, RA], f32)   # M diag-blocks b=0..7
            mb = mb_pool.tile([RB, HW], f32)   # M rows b=8..13 + 3 cam rows
            for ci in range(NCHUNK):
                st, sp = ci == 0, ci == NCHUNK - 1
                nc.tensor.matmul(
                    ma[:], lall[:, ci, 0:RA], lall[:, ci, 0:RA], start=st, stop=sp
                )
                nc.tensor.matmul(
                    mb[:], lall[:, ci, RA:199], lall[:, ci, 0:HW], start=st, stop=sp
                )
            # evacuate M to SBUF, batched across samples (engines need
            # quadrant-aligned partition starts; DMA gathers below don't)
            nc.scalar.copy(out=ma_all[:, s, :], in_=ma[:])
            nc.vector.tensor_copy(out=mb_all[:, s, :], in_=mb[:])

        # gathers: first half overlaps the tail of the main loop
        emit_cams(0, HH)
        emit_cams(HH, SH)
        emit_gdiag(0, HH, 0)
        emit_gdiag(HH, SH, 1)

        gall = singles.tile([14, SH * 14], f32)  # per-sample G side by side
        nc.vector.tensor_reduce(
            out=gall[:],
            in_=gdiag[:].rearrange("p s (b x) -> p s x b", x=14),
            axis=Ax.X, op=Alu.add,
        )

        # ---- batched CAM normalization: camn = (cam - min) / max(cam - min) * 255
        # rows 8..31 / 40..63 are zero padding; per-partition ops keep them inert
        mn = singles.tile([96, 1], f32)
        nc.vector.tensor_reduce(out=mn[:], in_=cams[:], axis=Ax.X, op=Alu.min)
        camsub = singles.tile([96, HW], f32)
        nc.vector.tensor_scalar(
            out=camsub[:], in0=cams[:], scalar1=mn[:], scalar2=None, op0=Alu.subtract
        )
        mx = singles.tile([96, 1], f32)
        nc.vector.tensor_reduce(out=mx[:], in_=camsub[:], axis=Ax.X, op=Alu.max)
        # keep the zero padding rows finite through the reciprocal
        nc.vector.tensor_scalar_max(out=mx[:], in0=mx[:], scalar1=1e-30)
        rmx = singles.tile([96, 1], f32)
        nc.vector.reciprocal(out=rmx[:], in_=mx[:])
        camn_wh = singles.tile([96, HW], f32)
        nc.vector.tensor_scalar(
            out=camn_wh[:], in0=camsub[:], scalar1=rmx[:], scalar2=255.0,
            op0=Alu.mult, op1=Alu.mult,
        )
        # cam rows came out (w,h)-ordered (matmul cols are lall-ordered);
        # one strided copy puts them in natural (h,w) order for everything
        # downstream (seg compare, row reduce, D reshape DMA)
        camn = singles.tile([96, HW], f32)
        nc.vector.tensor_copy(
            out=camn[:].rearrange("p (h w) -> p h w", w=14),
            in_=camn_wh[:].rearrange("p (w h) -> p h w", h=14),
        )

        # ---- D_t = cam0n - camtn, reshaped to [14,14] per sample in one DMA
        # (engine operands must share a partition range -> DMA-bounce the
        # cam1/cam2 quadrant blocks down to partitions 0..7 first)
        c1loc = singles.tile([SH, HW], f32)
        c2loc = singles.tile([SH, HW], f32)
        nc.sync.dma_start(out=c1loc[:], in_=camn[32:32 + SH, :])
        nc.sync.dma_start(out=c2loc[:], in_=camn[64:64 + SH, :])
        d12 = singles.tile([SH, 2 * HW], f32)
        nc.vector.tensor_tensor(
            out=d12[:, 0:HW], in0=camn[0:SH, :], in1=c1loc[:], op=Alu.subtract
        )
        nc.vector.tensor_tensor(
            out=d12[:, HW:2 * HW], in0=camn[0:SH, :], in1=c2loc[:], op=Alu.subtract
        )
        # dmats[i, (2s+t)*14 + j] = D_ts[i, j]  ((s,t) order keeps the source
        # AP mergeable to 3 dims: s stride 392 = 2x t stride 196)
        dmats = singles.tile([14, 2 * SH * 14], f32)
        nc.gpsimd.dma_start(
            out=dmats[:].rearrange("i (s t a) -> i s t a", s=SH, t=2),
            in_=d12[:].rearrange("s (t i a) -> i s t a", i=14, a=14),
        )

        # ---- ed1 (row-wise distance of binarized cam0 to seg truth)
        seg_sb = singles.tile([SH, HW], f32)
        nc.gpsimd.dma_start(out=seg_sb[:], in_=seg[:])
        x = singles.tile([SH, HW], f32)
        nc.vector.scalar_tensor_tensor(
            out=x[:], in0=camn[0:SH, :], scalar=THR, in1=seg_sb[:],
            op0=Alu.is_gt, op1=Alu.subtract,
        )  # x = (cam0n > THR) - seg
        eps_c = singles.tile([SH, 1], f32)
        nc.gpsimd.memset(eps_c[:], PD_EPS)
        xx = singles.tile([SH, HW], f32)
        nc.scalar.activation(out=xx[:], in_=x[:], func=Act.Square, bias=eps_c[:])
        r2 = singles.tile([SH, 14], f32)
        nc.vector.tensor_reduce(
            out=r2[:], in_=xx[:].rearrange("p (i a) -> p i a", a=14),
            axis=Ax.X, op=Alu.add,
        )
        rr = singles.tile([SH, 14], f32)
        nc.scalar.sqrt(rr[:], r2[:])
        ed1s = singles.tile([SH, 1], f32)
        nc.vector.tensor_reduce(out=ed1s[:], in_=rr[:], axis=Ax.X, op=Alu.add)

        # ---- cross entropy: lse(pred) - pred[cla]
        pred_sb = singles.tile([SH, NCLS], f32)
        nc.gpsimd.dma_start(out=pred_sb[:], in_=pred[:])
        cla_sb = singles.tile([SH, 1], i32)
        nc.gpsimd.dma_start(out=cla_sb[:], in_=cla[:])
        iot = singles.tile([SH, NCLS], f32)
        nc.gpsimd.iota(
            out=iot[:], pattern=[[1, NCLS]], base=0, channel_multiplier=0,
            allow_small_or_imprecise_dtypes=True,
        )
        cla_f = singles.tile([SH, 1], f32)
        nc.vector.tensor_copy(out=cla_f[:], in_=cla_sb[:])
        onehot = singles.tile([SH, NCLS], f32)
        nc.vector.tensor_scalar(
            out=onehot[:], in0=iot[:], scalar1=cla_f[:], scalar2=None,
            op0=Alu.is_equal,
        )
        scr1k = singles.tile([SH, NCLS], f32)
        nc.vector.tensor_mul(out=scr1k[:], in0=onehot[:], in1=pred_sb[:])
        tgt = singles.tile([SH, 1], f32)
        nc.vector.tensor_reduce(out=tgt[:], in_=scr1k[:], axis=Ax.X, op=Alu.add)
        pmax = singles.tile([SH, 1], f32)
        nc.vector.tensor_reduce(out=pmax[:], in_=pred_sb[:], axis=Ax.X, op=Alu.max)
        negm = singles.tile([SH, 1], f32)
        nc.vector.tensor_scalar(
            out=negm[:], in0=pmax[:], scalar1=-1.0, scalar2=None, op0=Alu.mult
        )
        esc = singles.tile([SH, NCLS], f32)
        sume = singles.tile([SH, 1], f32)
        nc.scalar.activation(
            out=esc[:], in_=pred_sb[:], func=Act.Exp, bias=negm[:], scale=1.0,
            accum_out=sume[:],
        )
        lns = singles.tile([SH, 1], f32)
        nc.scalar.activation(out=lns[:], in_=sume[:], func=Act.Ln)
        ce = singles.tile([SH, 1], f32)
        nc.vector.tensor_add(out=ce[:], in0=pmax[:], in1=lns[:])
        nc.vector.tensor_sub(out=ce[:], in0=ce[:], in1=tgt[:])

        # v = ed1s/14 + ce   (per-sample CE + seg-distance contribution)
        v = singles.tile([SH, 1], f32)
        nc.vector.scalar_tensor_tensor(
            out=v[:], in0=ed1s[:], scalar=1.0 / 14.0, in1=ce[:],
            op0=Alu.mult, op1=Alu.add,
        )

        # ---- C_ts = D_ts^T D_ts, all 16 into one PSUM tile; then
        # acc[i, k] = sum_x C_k[i, x] * G_s(k)[i, x] via one mul + one
        # grouped reduce (gall duplicated side-by-side to match k=(t,s))
        call = c_pool.tile([14, 2 * SH * 14], f32)
        for k in range(2 * SH):
            dsl = dmats[:, k * 14:(k + 1) * 14]
            nc.tensor.matmul(call[:, k * 14:(k + 1) * 14], dsl, dsl,
                             start=True, stop=True)
        gall2 = singles.tile([14, 2 * SH * 14], f32)
        gall2_v = gall2[:].rearrange("p (s t x) -> p s t x", t=2, x=14)
        gall_v = gall[:].rearrange("p (s x) -> p s x", x=14)
        nc.vector.tensor_copy(out=gall2_v[:, :, 0, :], in_=gall_v)
        nc.vector.tensor_copy(out=gall2_v[:, :, 1, :], in_=gall_v)
        prod = singles.tile([14, 2 * SH * 14], f32)
        nc.vector.tensor_mul(out=prod[:], in0=call[:], in1=gall2[:])
        acc = singles.tile([14, 2 * SH], f32)
        nc.vector.tensor_reduce(
            out=acc[:], in_=prod[:].rearrange("p (k x) -> p k x", x=14),
            axis=Ax.X, op=Alu.add,
        )

        # ---- partition-reduce acc + v via ones-matmuls, then the scalar tail
        ones = singles.tile([14, 1], f32)
        nc.gpsimd.memset(ones[:], 1.0)
        fs = fs_pool.tile([1, 2 * SH + 1], f32)
        nc.tensor.matmul(fs[:, 0:2 * SH], ones[:], acc[:], start=True, stop=True)
        nc.tensor.matmul(fs[:, 2 * SH:], ones[0:SH, :], v[:], start=True, stop=True)
        dvals = singles.tile([1, 2 * SH], f32)
        nc.scalar.activation(
            out=dvals[:], in_=fs[0:1, 0:2 * SH], func=Act.Sqrt,
            scale=1.0 / float(NCH) ** 2,
        )
        dv = dvals[:].rearrange("p (s t) -> p s t", t=2)
        dsum = singles.tile([1, SH], f32)
        nc.vector.tensor_tensor(
            out=dsum[:], in0=dv[:, :, 0], in1=dv[:, :, 1], op=Alu.add
        )
        marg_c = singles.tile([1, 1], f32)
        nc.gpsimd.memset(marg_c[:], MARGIN)
        relu_z = singles.tile([1, SH], f32)
        nc.scalar.activation(
            out=relu_z[:], in_=dsum[:], func=Act.Relu, bias=marg_c[:], scale=-1.0
        )
        rz = singles.tile([1, 1], f32)
        nc.vector.tensor_reduce(out=rz[:], in_=relu_z[:], axis=Ax.X, op=Alu.add)
        tot = singles.tile([1, 1], f32)
        nc.vector.tensor_add(out=tot[:], in0=rz[:], in1=fs[0:1, 2 * SH:2 * SH + 1])
        partial = singles.tile([1, 1], f32)
        nc.vector.tensor_scalar(
            out=partial[:], in0=tot[:], scalar1=1.0 / float(BZ), scalar2=None,
            op0=Alu.mult,
        )

        if collective:
            nc.sync.dma_start(out=cc_in[:], in_=partial[:])
            nc.gpsimd.collective_compute(
                "AllReduce",
                mybir.AluOpType.add,
                replica_groups=[list(range(NCORES))],
                ins=[cc_in[:]],
                outs=[cc_out[:]],
            )
            final_sb = singles.tile([1, 1], f32)
            nc.sync.dma_start(out=final_sb[:], in_=cc_out[:])
            nc.sync.dma_start(out=out_ext[:], in_=final_sb[:])
        else:
            nc.sync.dma_start(out=out_ext[:], in_=partial[:])

    return nc


USE_COLLECTIVE = False


def kernel(pred, cla_truth, seg_truth, features_blobs, weight_softmax, idx,
           _trace=False, _tmpdir=None):
    from concourse.bass_utils import run_bass_kernel_spmd

    if "nc" not in _CACHE:
        nc = _build(collective=USE_COLLECTIVE)
        if not nc.is_finalized():
            nc.finalize()
        _CACHE["nc"] = nc
    nc = _CACHE["nc"]

    pred = np.ascontiguousarray(np.asarray(pred, dtype=np.float32))
    cla = np.ascontiguousarray(np.asarray(cla_truth, dtype=np.int32))
    seg = np.ascontiguousarray(np.asarray(seg_truth, dtype=np.float32))
    feats = np.ascontiguousarray(np.asarray(features_blobs, dtype=np.float32))
    wsm = np.ascontiguousarray(np.asarray(weight_softmax, dtype=np.float32))
    idx = np.ascontiguousarray(np.asarray(idx, dtype=np.int32))

    in_maps = []
    for r in range(NCORES):
        sl = slice(r * SH, (r + 1) * SH)
        in_maps.append({
            "feats": np.ascontiguousarray(feats[sl]),
            "pred": np.ascontiguousarray(pred[sl]),
            "seg": np.ascontiguousarray(seg[sl].reshape(SH, HW)),
            "cla": np.ascontiguousarray(cla[sl].reshape(SH, 1)),
            "idx": np.ascontiguousarray(idx[sl].reshape(3 * SH, 1)),
            "wsm": wsm,
        })

    res = run_bass_kernel_spmd(
        nc, in_maps, list(range(NCORES)), trace=_trace, tmpdir=_tmpdir
    )
    if _trace:
        _CACHE["last_results"] = res
    if USE_COLLECTIVE:
        val = np.asarray(res.results[0]["out"]).reshape(())
    else:
        val = np.sum([np.asarray(r["out"]).reshape(()) for r in res.results],
                     dtype=np.float32)
    return np.float32(val)
